# revision 45
# baseline (speedup 1.0000x reference)
"""Trainium2 Bass kernel for nn_Attention_39573828665647.

GQA causal attention block (B=4, S=1024, DIM=2048, 32 q heads / 8 kv heads,
hd=64) with RoPE, sharded over 8 NeuronCores as (batch x head-half):
core = 2*b + hh handles batch b and kv groups [4hh, 4hh+4) (16 q heads).
Each core computes a partial output projection over its 1024 o-dims; the
host sums the two partials per batch.

On-device pipeline (all matmuls in fp32r = TF32):
  A: qkT = wqkv_slice @ x^T in transposed layout [heads*hd, s]; RoPE fused
     via stream_shuffle + 2 mul + add (interleaved pair rotation), q
     pre-scaled by 1/sqrt(hd) through the host-built cos/sin tables.
  B: v = x @ wv^T in natural layout [s, hd], augmented with a ones column
     so the attention matmul also produces softmax denominators.
  C: per head: S^T[sk,sq] = k^T.T @ q^T (PE), exp on ACT (no max
     subtraction - scores are O(5) bounded), causal handled by ragged
     tiles + a gpsimd affine_select on the diagonal chunk,
     o_aug[65,sq] = [v|1]^T @ expT accumulated over sk tiles.
  D: per-column normalization: reciprocal of the denominator row,
     partition-broadcast via a DRAM-bounce DMA.
  E: out[s,o] = o^T.T @ wo^T (wo prefetched during C), psum -> sbuf -> DRAM.
"""

import numpy as np

B, S, DIM = 4, 1024, 2048
NH, NKV, HD = 32, 8, 64
P = 128
ND = DIM // P  # 16 d-tiles

_SWAP_ADJ = [i ^ 1 for i in range(32)]  # pairwise partition swap within quadrants

_CACHE = {}


def host_prep(x, freqs_cos, freqs_sin, wqkv, wo):
    """Build the 8 per-core input dicts."""
    x = np.ascontiguousarray(np.asarray(x, np.float32))
    wqkv = np.asarray(wqkv, np.float32)
    wo = np.asarray(wo, np.float32)
    cos = np.asarray(freqs_cos, np.float32)
    sin = np.asarray(freqs_sin, np.float32)

    cosT, sinT = cos.T, sin.T                      # [32, S]
    C64 = np.repeat(cosT, 2, axis=0)               # [64, S]
    Ss64 = np.repeat(sinT, 2, axis=0).copy()
    Ss64[0::2] *= -1.0                             # even rows -sin, odd +sin
    C128 = np.ascontiguousarray(np.tile(C64, (2, 1)), dtype=np.float32)
    Ss128 = np.ascontiguousarray(np.tile(Ss64, (2, 1)), dtype=np.float32)
    scale = np.float32(1.0 / np.sqrt(HD))
    Cq, Sq = C128 * scale, Ss128 * scale
    Ck, Sk = C128, Ss128
    mask = np.triu(np.ones((P, P), np.float32))    # 1 where sq >= sk

    woT_full = np.ascontiguousarray(wo.T)          # [d', o]
    in_maps = []
    for core in range(8):
        b, hh = core // 2, core % 2
        groups = range(4 * hh, 4 * hh + 4)
        qheads = range(16 * hh, 16 * hh + 16)
        q_rows = np.concatenate(
            [np.arange((h // 4 * 6 + h % 4) * 64, (h // 4 * 6 + h % 4) * 64 + 64)
             for h in qheads])
        k_rows = np.concatenate(
            [np.arange((g * 6 + 4) * 64, (g * 6 + 4) * 64 + 64) for g in groups])
        v_rows = np.concatenate(
            [np.arange((g * 6 + 5) * 64, (g * 6 + 5) * 64 + 64) for g in groups])
        rows = np.concatenate([q_rows, k_rows, v_rows])
        in_maps.append({
            "xT": np.ascontiguousarray(x[b].T),                    # [2048, 1024]
            "wqkvT": np.ascontiguousarray(wqkv[rows].T),           # [2048, 1536]
            "woT": np.ascontiguousarray(woT_full[1024 * hh:1024 * hh + 1024]),
            "Cq": Cq, "Sq": Sq, "Ck": Ck, "Sk": Sk, "mask": mask,
        })
    return in_maps


def build_nc(reps=1):
    from contextlib import ExitStack
    import concourse.bacc as bacc
    import concourse.bass as bass
    import concourse.tile as tile
    import concourse.mybir as mybir

    f32 = mybir.dt.float32
    f32r = mybir.dt.float32r
    EXP = mybir.ActivationFunctionType.Exp

    nc = bacc.Bacc("TRN2", target_bir_lowering=False, debug=False)
    xT_d = nc.dram_tensor("xT", [DIM, S], f32r, kind="ExternalInput")
    wqkvT_d = nc.dram_tensor("wqkvT", [DIM, 1536], f32r, kind="ExternalInput")
    woT_d = nc.dram_tensor("woT", [1024, DIM], f32r, kind="ExternalInput")
    Cq_d = nc.dram_tensor("Cq", [P, S], f32, kind="ExternalInput")
    Sq_d = nc.dram_tensor("Sq", [P, S], f32, kind="ExternalInput")
    Ck_d = nc.dram_tensor("Ck", [P, S], f32, kind="ExternalInput")
    Sk_d = nc.dram_tensor("Sk", [P, S], f32, kind="ExternalInput")
    out_d = nc.dram_tensor("out", [S, DIM], f32, kind="ExternalOutput")

    def emit(tc, pfx):
        with ExitStack() as stack:
            resid = stack.enter_context(tc.tile_pool(name=pfx + "resid", bufs=1))

            def rtile(shape, dt_, nm):
                return resid.tile(shape, dt_, tag=pfx + nm, name=pfx + nm)

            q_sb = [rtile([P, S], f32r, f"q{i}") for i in range(8)]
            k_sb = [rtile([P, S], f32r, f"k{g}") for g in range(4)]
            vaug = [rtile([P, 4, 65], f32r, f"va{i}") for i in range(8)]

            # ------------- Stage A + B: projections + rope -------------
            with tc.tile_pool(name=pfx + "xres", bufs=1) as xres_pool, \
                 tc.tile_pool(name=pfx + "ropeconst", bufs=1) as rc_pool, \
                 tc.tile_pool(name=pfx + "wstream", bufs=1) as w_pool, \
                 tc.tile_pool(name=pfx + "ropetmp", bufs=3) as rt_pool, \
                 tc.tile_pool(name=pfx + "psumA", bufs=8, space="PSUM") as psA:

                # PE warmup: spin matmuls on const data while the first
                # DMAs land, so HAM un-throttles and PE isn't idle. Uses a
                # psA slot so nothing downstream waits on a pool release.
                wmt = rt_pool.tile([P, P], f32r, tag="sh", name=pfx + "wm")
                nc.vector.tensor_copy(wmt[:], nc.const_aps.tensor(0.0, (P, P), f32))
                wps = psA.tile([P, P], f32, tag="acc", name=pfx + "wps")
                for _ in range(26):
                    nc.tensor.matmul(wps[:], wmt[:], wmt[:], start=True, stop=True)

                xres, wq = [], []
                c_sb = {}
                for d in range(ND):
                    xt = xres_pool.tile([P, S], f32r, tag=f"x{d}",
                                        name=pfx + f"x{d}")
                    wt = w_pool.tile([P, 512], f32r, tag=f"wq{d}",
                                     name=pfx + f"wq{d}")
                    eng_a = nc.sync if d % 2 == 0 else nc.scalar
                    eng_b = nc.scalar if d % 2 == 0 else nc.sync
                    # halves on opposite queues: t=0 matmuls start sooner
                    eng_a.dma_start(out=xt[:, 0:512],
                                    in_=xT_d[d * P:(d + 1) * P, 0:512])
                    eng_b.dma_start(out=wt[:],
                                    in_=wqkvT_d[d * P:(d + 1) * P, 0:512])
                    eng_a.dma_start(out=xt[:, 512:S],
                                    in_=xT_d[d * P:(d + 1) * P, 512:S])
                    xres.append(xt)
                    wq.append(wt)
                    if d == 5:  # rope tables: needed from the first rope on
                        for i, (nm, dr) in enumerate(
                                (("Cq", Cq_d), ("Sq", Sq_d),
                                 ("Ck", Ck_d), ("Sk", Sk_d))):
                            ct = rc_pool.tile([P, S], f32, tag=nm, name=pfx + nm)
                            (nc.sync if i % 2 else nc.scalar).dma_start(
                                out=ct[:], in_=dr[:])
                            c_sb[nm] = ct

                def rope(ptile, at, t):
                    sl = slice(t * 512, t * 512 + 512)
                    is_q = at < 8
                    C_ = c_sb["Cq" if is_q else "Ck"]
                    S_ = c_sb["Sq" if is_q else "Sk"]
                    sh = rt_pool.tile([P, 512], f32, tag="sh",
                                      name=pfx + f"sh{at}_{t}")
                    m1 = rt_pool.tile([P, 512], f32, tag="m1",
                                      name=pfx + f"m1_{at}_{t}")
                    m2 = rt_pool.tile([P, 512], f32, tag="m2",
                                      name=pfx + f"m2_{at}_{t}")
                    nc.vector.stream_shuffle(sh[:], ptile[:], _SWAP_ADJ)
                    nc.vector.tensor_mul(m1[:], ptile[:], C_[:, sl])
                    nc.gpsimd.tensor_mul(m2[:], sh[:], S_[:, sl])
                    if is_q:
                        nc.gpsimd.tensor_add(q_sb[at][:, sl], m1[:], m2[:])
                    else:
                        ro = rt_pool.tile([P, 512], f32r, tag="ro",
                                          name=pfx + f"ro{at}_{t}")
                        nc.gpsimd.tensor_add(ro[:], m1[:], m2[:])
                        for half in (0, 1):
                            g = 2 * (at - 8) + half
                            src = ro[half * 64:half * 64 + 64, :]
                            nc.scalar.copy(k_sb[g][0:64, sl], src)
                            nc.scalar.copy(k_sb[g][64:128, sl], src)

                def qk_group(ats, wtiles, coff):
                    for t in (0, 1):
                        for at in ats:
                            pt = psA.tile([P, 512], f32, tag="acc",
                                          name=pfx + f"acc{at}_{t}")
                            for d in range(ND):
                                nc.tensor.matmul(
                                    pt[:],
                                    wtiles[d][:, (at - coff) * P:(at - coff + 1) * P],
                                    xres[d][:, t * 512:(t + 1) * 512],
                                    start=(d == 0), stop=(d == ND - 1))
                            rope(pt, at, t)

                qk_group(range(4), wq, 0)

                wq2 = []
                for d in range(ND):
                    wt = w_pool.tile([P, 512], f32r, tag=f"wq{d}",
                                     name=pfx + f"w2_{d}")
                    (nc.sync if d % 2 else nc.scalar).dma_start(
                        out=wt[:], in_=wqkvT_d[d * P:(d + 1) * P, 512:1024])
                    wq2.append(wt)
                qk_group(range(4, 8), wq2, 4)

                wkv = []
                for d in range(ND):
                    wt = w_pool.tile([P, 512], f32r, tag=f"wq{d}",
                                     name=pfx + f"w3_{d}")
                    (nc.sync if d % 2 else nc.scalar).dma_start(
                        out=wt[:], in_=wqkvT_d[d * P:(d + 1) * P, 1024:1536])
                    wkv.append(wt)
                qk_group((8, 9), wkv, 8)

                # Stage B: v projection (natural layout) + ones augmentation
                for st in range(8):
                    pt = psA.tile([P, 256], f32, tag="acc", name=pfx + f"vacc{st}")
                    for d in range(ND):
                        nc.tensor.matmul(
                            pt[:], xres[d][:, st * P:(st + 1) * P],
                            wkv[d][:, 256:512], start=(d == 0), stop=(d == ND - 1))
                    nc.vector.tensor_copy(vaug[st][:, :, 64],
                                          nc.const_aps.tensor(1.0, (P, 4), f32))
                    for g in range(4):
                        nc.scalar.copy(
                            vaug[st][:, g, 0:64], pt[:, g * 64:(g + 1) * 64])

            # ------------- Stage C + D: attention -------------
            o_pool = stack.enter_context(tc.tile_pool(name=pfx + "opool", bufs=1))
            o_sb = [o_pool.tile([P, S], f32r, tag=f"o{i}", name=pfx + f"o{i}")
                    for i in range(8)]
            wo_pool = stack.enter_context(tc.tile_pool(name=pfx + "wo", bufs=1))
            wo_sb = {}

            def load_wo(i):
                ot, dt_ = i // 8, i % 8
                w = wo_pool.tile([P, 512], f32r, tag=f"wo{ot}_{dt_}",
                                 name=pfx + f"wo{ot}_{dt_}")
                nc.scalar.dma_start(
                    out=w[:],
                    in_=woT_d[dt_ * P:(dt_ + 1) * P, ot * 512:(ot + 1) * 512])
                wo_sb[(ot, dt_)] = w

            with tc.tile_pool(name=pfx + "expT", bufs=6) as e_pool, \
                 tc.tile_pool(name=pfx + "rdram", bufs=4, space="DRAM") as rd_pool, \
                 tc.tile_pool(name=pfx + "normtmp", bufs=4) as n_pool, \
                 tc.tile_pool(name=pfx + "outsb", bufs=4) as ob_pool, \
                 tc.tile_pool(name=pfx + "psumS", bufs=2, space="PSUM") as psS, \
                 tc.tile_pool(name=pfx + "psumO", bufs=4, space="PSUM") as psO:

                def normalize(h, t, opsum):
                    par = h % 2
                    r = n_pool.tile([1, 512], f32, tag="r", name=pfx + f"r{h}_{t}")
                    nc.vector.reciprocal(r[:], opsum[64:65, :])
                    rd = rd_pool.tile([1, 512], f32, tag="rd",
                                      name=pfx + f"rd{h}_{t}")
                    nc.sync.dma_start(out=rd[:], in_=r[:])
                    rb = n_pool.tile([64, 512], f32, tag="rb",
                                     name=pfx + f"rb{h}_{t}")
                    rdap = rd[:]
                    bcast = bass.AP(tensor=rdap.tensor, offset=rdap.offset,
                                    ap=[[0, 64]] + [list(p) for p in rdap.ap[1:]])
                    nc.sync.dma_start(out=rb[:], in_=bcast)
                    dst = o_sb[h // 2][par * 64:par * 64 + 64,
                                       t * 512:(t + 1) * 512]
                    nc.vector.tensor_mul(dst, opsum[0:64, :], rb[:])

                for h in range(16):
                    g = h // 4
                    par = h % 2
                    qh = q_sb[h // 2][par * 64:par * 64 + 64, :]
                    kh = k_sb[g][par * 64:par * 64 + 64, :]
                    opsum = [psO.tile([65, 512], f32, tag="op",
                                      name=pfx + f"op{h}_{t}") for t in (0, 1)]
                    for j in range(8):
                        lo = j * P
                        spsum = psS.tile([P, S], f32, tag="sp",
                                         name=pfx + f"sp{h}_{j}")
                        if j < 4:  # first sq-half (ragged)
                            nc.tensor.matmul(
                                spsum[:, lo:512], kh[:, j * P:(j + 1) * P],
                                qh[:, lo:512], start=True, stop=True)
                        nc.tensor.matmul(
                            spsum[:, 512:S], kh[:, j * P:(j + 1) * P],
                            qh[:, 512:S], start=True, stop=True)
                        et = e_pool.tile([P, S], f32r, tag="et",
                                         name=pfx + f"et{h}_{j}")
                        nc.scalar.activation(et[:, lo:S], spsum[:, lo:S], EXP)
                        # diagonal chunk: zero sq < sk
                        nc.gpsimd.affine_select(
                            out=et[:, lo:lo + P], in_=et[:, lo:lo + P],
                            pattern=[[1, P]], channel_multiplier=-1,
                            base=0, compare_op=mybir.AluOpType.is_ge, fill=0.0)
                        if j < 4:
                            nc.tensor.matmul(
                                opsum[0][0:65, lo:512], vaug[j][:, g, :],
                                et[:, lo:512], start=(j == 0), stop=(j == 3))
                        lo1 = max(lo - 512, 0)
                        nc.tensor.matmul(
                            opsum[1][0:65, lo1:512], vaug[j][:, g, :],
                            et[:, 512 + lo1:S], start=(j == 0), stop=(j == 7))
                        if j == 3:
                            normalize(h, 0, opsum[0])
                    normalize(h, 1, opsum[1])
                    load_wo(2 * h)
                    load_wo(2 * h + 1)

                # ------------- Stage E: output projection -------------
                # pe tiles share psO's "op" slots: stage E acquires banks at
                # slot granularity as the last heads' opsums retire (no
                # pool-level barrier between attention and the projection).
                for ot in range(4):
                    for sc in range(8):
                        pe = psO.tile([P, 512], f32, tag="op",
                                      name=pfx + f"pe{ot}_{sc}")
                        for dt_ in range(8):
                            nc.tensor.matmul(
                                pe[:], o_sb[dt_][:, sc * P:(sc + 1) * P],
                                wo_sb[(ot, dt_)][:],
                                start=(dt_ == 0), stop=(dt_ == 7))
                        ob = ob_pool.tile([P, 512], f32, tag="ob",
                                          name=pfx + f"ob{ot}_{sc}")
                        nc.vector.tensor_copy(ob[:], pe[:])
                        (nc.sync if sc % 2 else nc.scalar).dma_start(
                            out=out_d[sc * P:(sc + 1) * P,
                                      ot * 512:(ot + 1) * 512],
                            in_=ob[:])

    with tile.TileContext(nc) as tc:
        for rep in range(reps):
            emit(tc, f"r{rep}_" if reps > 1 else "")

    nc.compile()
    return nc


def _get_nc():
    if "nc" not in _CACHE:
        _CACHE["nc"] = build_nc()
    return _CACHE["nc"]


def kernel(**inputs):
    from concourse.bass_utils import run_bass_kernel_spmd
    nc = _get_nc()
    in_maps = host_prep(**inputs)
    res = run_bass_kernel_spmd(nc, in_maps, core_ids=list(range(8)))
    outs = [res.results[c]["out"] for c in range(8)]
    full = np.stack([outs[2 * b] + outs[2 * b + 1] for b in range(B)])
    return full.astype(np.float32)


if __name__ == "__main__":
    nc = build_nc()
    print("build ok")


# revision 60
# speedup vs baseline: 1.1933x; 1.1933x over previous
"""Trainium2 Bass kernel for nn_Attention_39573828665647.

GQA causal attention block (B=4, S=1024, DIM=2048, 32 q heads / 8 kv heads,
hd=64) with RoPE, sharded over 8 NeuronCores as (batch x head-half):
core = 2*b + hh handles batch b and kv groups [4hh, 4hh+4) (16 q heads).
Each core computes a partial output projection over its 1024 o-dims; the
host sums the two partials per batch.

On-device pipeline (all matmuls in fp32r = TF32):
  A: qkT = wqkv_slice @ x^T in transposed layout [heads*hd, s]; RoPE fused
     via stream_shuffle + 2 mul + add (interleaved pair rotation), q
     pre-scaled by 1/sqrt(hd) through the host-built cos/sin tables.
  B: v = x @ wv^T in natural layout [s, hd], augmented with a ones column
     so the attention matmul also produces softmax denominators.
  C: per head: S^T[sk,sq] = k^T.T @ q^T (PE), exp on ACT (no max
     subtraction - scores are O(5) bounded), causal handled by ragged
     tiles + a gpsimd affine_select on the diagonal chunk,
     o_aug[65,sq] = [v|1]^T @ expT accumulated over sk tiles.
  D: per-column normalization: reciprocal of the denominator row,
     partition-broadcast via a DRAM-bounce DMA.
  E: out[s,o] = o^T.T @ wo^T (wo prefetched during C), psum -> sbuf -> DRAM.
"""

import numpy as np

B, S, DIM = 4, 1024, 2048
NH, NKV, HD = 32, 8, 64
P = 128
ND = DIM // P  # 16 d-tiles

_SWAP_ADJ = [i ^ 1 for i in range(32)]  # pairwise partition swap within quadrants

_CACHE = {}


def host_prep(x, freqs_cos, freqs_sin, wqkv, wo):
    """Build the 8 per-core input dicts."""
    x = np.ascontiguousarray(np.asarray(x, np.float32))
    wqkv = np.asarray(wqkv, np.float32)
    wo = np.asarray(wo, np.float32)
    cos = np.asarray(freqs_cos, np.float32)
    sin = np.asarray(freqs_sin, np.float32)

    cosT, sinT = cos.T, sin.T                      # [32, S]
    C64 = np.repeat(cosT, 2, axis=0)               # [64, S]
    Ss64 = np.repeat(sinT, 2, axis=0).copy()
    Ss64[0::2] *= -1.0                             # even rows -sin, odd +sin
    C64 = np.ascontiguousarray(C64, dtype=np.float32)
    Ss64 = np.ascontiguousarray(Ss64, dtype=np.float32)
    scale = np.float32(1.0 / np.sqrt(HD))
    Cq, Sq = C64 * scale, Ss64 * scale      # [64, S]; kernel duplicates rows
    Ck, Sk = C64, Ss64
    mask = np.triu(np.ones((P, P), np.float32))    # 1 where sq >= sk

    woT_full = np.ascontiguousarray(wo.T)          # [d', o]
    xT_full = np.ascontiguousarray(x.transpose(0, 2, 1))  # [B, DIM, S]
    wqkvT_full = np.ascontiguousarray(wqkv.T)      # [DIM, 3072]
    in_maps = []
    for core in range(8):
        b, hh = core // 2, core % 2
        groups = range(4 * hh, 4 * hh + 4)
        qheads = range(16 * hh, 16 * hh + 16)
        # assemble wqkvT from contiguous 64-column blocks (memcpy-speed)
        wqkvT = np.empty((DIM, 1536), np.float32)
        col = 0
        blocks = ([(h // 4 * 6 + h % 4) * 64 for h in qheads]
                  + [(g * 6 + 4) * 64 for g in groups]
                  + [(g * 6 + 5) * 64 for g in groups])
        for c0 in blocks:
            wqkvT[:, col:col + 64] = wqkvT_full[:, c0:c0 + 64]
            col += 64
        in_maps.append({
            "xT": xT_full[b],                                      # [2048, 1024]
            "wqkvT": wqkvT,                                        # [2048, 1536]
            "woT": np.ascontiguousarray(woT_full[1024 * hh:1024 * hh + 1024]),
            "Cq": Cq, "Sq": Sq, "Ck": Ck, "Sk": Sk, "mask": mask,
        })
    return in_maps


def build_nc(reps=1):
    from contextlib import ExitStack
    import concourse.bacc as bacc
    import concourse.bass as bass
    import concourse.tile as tile
    import concourse.mybir as mybir

    f32 = mybir.dt.float32
    f32r = mybir.dt.float32r
    EXP = mybir.ActivationFunctionType.Exp

    nc = bacc.Bacc("TRN2", target_bir_lowering=False, debug=False)
    xT_d = nc.dram_tensor("xT", [DIM, S], f32r, kind="ExternalInput")
    wqkvT_d = nc.dram_tensor("wqkvT", [DIM, 1536], f32r, kind="ExternalInput")
    woT_d = nc.dram_tensor("woT", [1024, DIM], f32r, kind="ExternalInput")
    Cq_d = nc.dram_tensor("Cq", [64, S], f32, kind="ExternalInput")
    Sq_d = nc.dram_tensor("Sq", [64, S], f32, kind="ExternalInput")
    Ck_d = nc.dram_tensor("Ck", [64, S], f32, kind="ExternalInput")
    Sk_d = nc.dram_tensor("Sk", [64, S], f32, kind="ExternalInput")
    out_d = nc.dram_tensor("out", [S, DIM], f32, kind="ExternalOutput")

    def emit(tc, pfx):
        with ExitStack() as stack:
            resid = stack.enter_context(tc.tile_pool(name=pfx + "resid", bufs=1))

            def rtile(shape, dt_, nm):
                return resid.tile(shape, dt_, tag=pfx + nm, name=pfx + nm)

            q_sb = [rtile([P, S], f32r, f"q{i}") for i in range(8)]
            k_sb = [rtile([P, S], f32r, f"k{g}") for g in range(4)]
            vaug = [rtile([P, 4, 65], f32r, f"va{i}") for i in range(8)]

            # ------------- Stage A + B: projections + rope -------------
            with tc.tile_pool(name=pfx + "xres", bufs=1) as xres_pool, \
                 tc.tile_pool(name=pfx + "ropeconst", bufs=1) as rc_pool, \
                 tc.tile_pool(name=pfx + "wstream", bufs=1) as w_pool, \
                 tc.tile_pool(name=pfx + "ropetmp", bufs=3) as rt_pool, \
                 tc.tile_pool(name=pfx + "psumA", bufs=8, space="PSUM") as psA:

                # PE warmup: spin matmuls on const data while the first
                # DMAs land, so HAM un-throttles and PE isn't idle. Uses a
                # psA slot so nothing downstream waits on a pool release.
                wmt = rt_pool.tile([P, P], f32r, tag="sh", name=pfx + "wm")
                nc.vector.tensor_copy(wmt[:], nc.const_aps.tensor(0.0, (P, P), f32))
                wps = psA.tile([P, P], f32, tag="acc", name=pfx + "wps")
                for _ in range(26):
                    nc.tensor.matmul(wps[:], wmt[:], wmt[:], start=True, stop=True)

                xres, wq = [], []
                c_sb = {}
                for d in range(ND):
                    xt = xres_pool.tile([P, S], f32r, tag=f"x{d}",
                                        name=pfx + f"x{d}")
                    wt = w_pool.tile([P, 512], f32r, tag=f"wq{d}",
                                     name=pfx + f"wq{d}")
                    eng_a = nc.sync if d % 2 == 0 else nc.scalar
                    eng_b = nc.scalar if d % 2 == 0 else nc.sync
                    # halves on opposite queues: t=0 matmuls start sooner
                    eng_a.dma_start(out=xt[:, 0:512],
                                    in_=xT_d[d * P:(d + 1) * P, 0:512])
                    eng_b.dma_start(out=wt[:],
                                    in_=wqkvT_d[d * P:(d + 1) * P, 0:512])
                    eng_a.dma_start(out=xt[:, 512:S],
                                    in_=xT_d[d * P:(d + 1) * P, 512:S])
                    xres.append(xt)
                    wq.append(wt)
                    if d == 10:  # rope tables (rows 64-127 duplicate 0-63:
                        # DMA half, duplicate on the idle ACT engine - halves
                        # the tables' claim on the DMA-fill window)
                        for i, (nm, dr) in enumerate(
                                (("Cq", Cq_d), ("Sq", Sq_d),
                                 ("Ck", Ck_d), ("Sk", Sk_d))):
                            ct = rc_pool.tile([P, S], f32, tag=nm, name=pfx + nm)
                            (nc.sync if i % 2 else nc.scalar).dma_start(
                                out=ct[0:64, :], in_=dr[:])
                            nc.scalar.copy(ct[64:128, :], ct[0:64, :])
                            c_sb[nm] = ct

                def rope(ptile, at, t):
                    sl = slice(t * 512, t * 512 + 512)
                    is_q = at < 8
                    C_ = c_sb["Cq" if is_q else "Ck"]
                    S_ = c_sb["Sq" if is_q else "Sk"]
                    sh = rt_pool.tile([P, 512], f32, tag="sh",
                                      name=pfx + f"sh{at}_{t}")
                    m1 = rt_pool.tile([P, 512], f32, tag="m1",
                                      name=pfx + f"m1_{at}_{t}")
                    m2 = rt_pool.tile([P, 512], f32, tag="m2",
                                      name=pfx + f"m2_{at}_{t}")
                    nc.vector.stream_shuffle(sh[:], ptile[:], _SWAP_ADJ)
                    nc.vector.tensor_mul(m1[:], ptile[:], C_[:, sl])
                    nc.gpsimd.tensor_mul(m2[:], sh[:], S_[:, sl])
                    if is_q:
                        nc.gpsimd.tensor_add(q_sb[at][:, sl], m1[:], m2[:])
                    else:
                        ro = rt_pool.tile([P, 512], f32r, tag="ro",
                                          name=pfx + f"ro{at}_{t}")
                        nc.gpsimd.tensor_add(ro[:], m1[:], m2[:])
                        for half in (0, 1):
                            g = 2 * (at - 8) + half
                            src = ro[half * 64:half * 64 + 64, :]
                            nc.scalar.copy(k_sb[g][0:64, sl], src)
                            nc.scalar.copy(k_sb[g][64:128, sl], src)

                def qk_group(ats, wtiles, coff):
                    for t in (0, 1):
                        for at in ats:
                            pt = psA.tile([P, 512], f32, tag="acc",
                                          name=pfx + f"acc{at}_{t}")
                            for d in range(ND):
                                nc.tensor.matmul(
                                    pt[:],
                                    wtiles[d][:, (at - coff) * P:(at - coff + 1) * P],
                                    xres[d][:, t * 512:(t + 1) * 512],
                                    start=(d == 0), stop=(d == ND - 1))
                            rope(pt, at, t)

                qk_group(range(4), wq, 0)

                wq2 = []
                for d in range(ND):
                    wt = w_pool.tile([P, 512], f32r, tag=f"wq{d}",
                                     name=pfx + f"w2_{d}")
                    (nc.sync if d % 2 else nc.scalar).dma_start(
                        out=wt[:], in_=wqkvT_d[d * P:(d + 1) * P, 512:1024])
                    wq2.append(wt)
                qk_group(range(4, 8), wq2, 4)

                wkv = []
                for d in range(ND):
                    wt = w_pool.tile([P, 512], f32r, tag=f"wq{d}",
                                     name=pfx + f"w3_{d}")
                    (nc.sync if d % 2 else nc.scalar).dma_start(
                        out=wt[:], in_=wqkvT_d[d * P:(d + 1) * P, 1024:1536])
                    wkv.append(wt)
                qk_group((8, 9), wkv, 8)

                # Stage B: v projection (natural layout) + ones augmentation
                for st in range(8):
                    pt = psA.tile([P, 256], f32, tag="acc", name=pfx + f"vacc{st}")
                    for d in range(ND):
                        nc.tensor.matmul(
                            pt[:], xres[d][:, st * P:(st + 1) * P],
                            wkv[d][:, 256:512], start=(d == 0), stop=(d == ND - 1))
                    nc.vector.tensor_copy(vaug[st][:, :, 64],
                                          nc.const_aps.tensor(1.0, (P, 4), f32))
                    for g in range(4):
                        nc.scalar.copy(
                            vaug[st][:, g, 0:64], pt[:, g * 64:(g + 1) * 64])

            # ------------- Stage C + D: attention -------------
            o_pool = stack.enter_context(tc.tile_pool(name=pfx + "opool", bufs=1))
            o_sb = [o_pool.tile([P, S], f32r, tag=f"o{i}", name=pfx + f"o{i}")
                    for i in range(8)]
            wo_pool = stack.enter_context(tc.tile_pool(name=pfx + "wo", bufs=1))
            wo_sb = {}

            def load_wo(i):
                ot, dt_ = i // 8, i % 8
                w = wo_pool.tile([P, 512], f32r, tag=f"wo{ot}_{dt_}",
                                 name=pfx + f"wo{ot}_{dt_}")
                nc.scalar.dma_start(
                    out=w[:],
                    in_=woT_d[dt_ * P:(dt_ + 1) * P, ot * 512:(ot + 1) * 512])
                wo_sb[(ot, dt_)] = w

            with tc.tile_pool(name=pfx + "expT", bufs=6) as e_pool, \
                 tc.tile_pool(name=pfx + "rdram", bufs=4, space="DRAM") as rd_pool, \
                 tc.tile_pool(name=pfx + "normtmp", bufs=4) as n_pool, \
                 tc.tile_pool(name=pfx + "outsb", bufs=4) as ob_pool, \
                 tc.tile_pool(name=pfx + "psumS", bufs=2, space="PSUM") as psS, \
                 tc.tile_pool(name=pfx + "psumO", bufs=4, space="PSUM") as psO:

                def normalize(h, t, opsum):
                    par = h % 2
                    r = n_pool.tile([1, 512], f32, tag="r", name=pfx + f"r{h}_{t}")
                    nc.vector.reciprocal(r[:], opsum[64:65, :])
                    rd = rd_pool.tile([1, 512], f32, tag="rd",
                                      name=pfx + f"rd{h}_{t}")
                    nc.sync.dma_start(out=rd[:], in_=r[:])
                    rb = n_pool.tile([64, 512], f32, tag="rb",
                                     name=pfx + f"rb{h}_{t}")
                    rdap = rd[:]
                    bcast = bass.AP(tensor=rdap.tensor, offset=rdap.offset,
                                    ap=[[0, 64]] + [list(p) for p in rdap.ap[1:]])
                    nc.sync.dma_start(out=rb[:], in_=bcast)
                    dst = o_sb[h // 2][par * 64:par * 64 + 64,
                                       t * 512:(t + 1) * 512]
                    nc.vector.tensor_mul(dst, opsum[0:64, :], rb[:])

                for h in range(16):
                    g = h // 4
                    par = h % 2
                    qh = q_sb[h // 2][par * 64:par * 64 + 64, :]
                    kh = k_sb[g][par * 64:par * 64 + 64, :]
                    opsum = [psO.tile([65, 512], f32, tag="op",
                                      name=pfx + f"op{h}_{t}") for t in (0, 1)]
                    for j in range(8):
                        lo = j * P
                        spsum = psS.tile([P, S], f32, tag="sp",
                                         name=pfx + f"sp{h}_{j}")
                        if j < 4:  # first sq-half (ragged)
                            nc.tensor.matmul(
                                spsum[:, lo:512], kh[:, j * P:(j + 1) * P],
                                qh[:, lo:512], start=True, stop=True)
                        hi = max(lo, 512)  # second half ragged too: only
                        nc.tensor.matmul(  # sq >= sk columns are needed
                            spsum[:, hi:S], kh[:, j * P:(j + 1) * P],
                            qh[:, hi:S], start=True, stop=True)
                        et = e_pool.tile([P, S], f32r, tag="et",
                                         name=pfx + f"et{h}_{j}")
                        nc.scalar.activation(et[:, lo:S], spsum[:, lo:S], EXP)
                        # diagonal chunk: zero sq < sk
                        nc.gpsimd.affine_select(
                            out=et[:, lo:lo + P], in_=et[:, lo:lo + P],
                            pattern=[[1, P]], channel_multiplier=-1,
                            base=0, compare_op=mybir.AluOpType.is_ge, fill=0.0)
                        if j < 4:
                            nc.tensor.matmul(
                                opsum[0][0:65, lo:512], vaug[j][:, g, :],
                                et[:, lo:512], start=(j == 0), stop=(j == 3))
                        lo1 = max(lo - 512, 0)
                        nc.tensor.matmul(
                            opsum[1][0:65, lo1:512], vaug[j][:, g, :],
                            et[:, 512 + lo1:S], start=(j == 0), stop=(j == 7))
                        if j == 3:
                            normalize(h, 0, opsum[0])
                    normalize(h, 1, opsum[1])
                    load_wo(2 * h)
                    load_wo(2 * h + 1)

                # ------------- Stage E: output projection -------------
                # pe tiles share psO's "op" slots: stage E acquires banks at
                # slot granularity as the last heads' opsums retire (no
                # pool-level barrier between attention and the projection).
                for ot in range(4):
                    for sc in range(8):
                        pe = psO.tile([P, 512], f32, tag="op",
                                      name=pfx + f"pe{ot}_{sc}")
                        for dt_ in range(8):
                            nc.tensor.matmul(
                                pe[:], o_sb[dt_][:, sc * P:(sc + 1) * P],
                                wo_sb[(ot, dt_)][:],
                                start=(dt_ == 0), stop=(dt_ == 7))
                        ob = ob_pool.tile([P, 512], f32, tag="ob",
                                          name=pfx + f"ob{ot}_{sc}")
                        nc.vector.tensor_copy(ob[:], pe[:])
                        (nc.sync if sc % 2 else nc.scalar).dma_start(
                            out=out_d[sc * P:(sc + 1) * P,
                                      ot * 512:(ot + 1) * 512],
                            in_=ob[:])

    with tile.TileContext(nc) as tc:
        for rep in range(reps):
            emit(tc, f"r{rep}_" if reps > 1 else "")

    nc.compile()
    return nc


def _get_nc():
    if "nc" not in _CACHE:
        _CACHE["nc"] = build_nc()
    return _CACHE["nc"]


def kernel(**inputs):
    from concourse.bass_utils import run_bass_kernel_spmd
    nc = _get_nc()
    in_maps = host_prep(**inputs)
    res = run_bass_kernel_spmd(nc, in_maps, core_ids=list(range(8)))
    outs = [res.results[c]["out"] for c in range(8)]
    full = np.stack([outs[2 * b] + outs[2 * b + 1] for b in range(B)])
    return full.astype(np.float32)


if __name__ == "__main__":
    nc = build_nc()
    print("build ok")
